# revision 6
# baseline (speedup 1.0000x reference)
"""Trainium2 Bass kernel for AttentionAggregator, v2 (descriptor-minimal).

Reference computation:
    new_emb = fb @ W + b
    s_e     = (fa @ a1)[src_e] + (new_emb @ a2)[dst_e]
    score_e = exp(elu(s_e, 0.1))
    out[n]  = (sum_{e: src_e=n} score_e * new_emb[dst_e]) / max(den[n], 1)

Reformulation:
    q_m   = fb[m] @ (W @ a2);  c1_n = fa[n] @ a1 + b @ a2
    s_e   = c1[src_e] + q[dst_e]
    score = exp(elu(s)) = max(e^s, 0.9 + 0.1 e^s)        (exact identity)
    G[n]  = sum_e score_e fb[dst_e];  den[n] = sum_e score_e
    out[n]= (G[n]/den_safe) @ W + [den>0] * b

Device strategy (per core, nodes sharded contiguously):
  - edges sorted by src, grouped into 64-node windows, padded to 128-edge
    tiles; raw f32 fb rows gathered per 1024-idx call into rotating tiles
    with 4-queue dma_gather (1 x 256B descriptor per edge, the dominant
    cost); q, the bf16 cast and a ones column are derived on device
  - a one-hot matrix Seg[e,n] = [src_e == n] * score_e feeds PE matmuls
    that segment-sum score*[fb|1] directly into a [64,65] PSUM
    accumulator per window
  - flush: divide by den, then one PE matmul [hT|mask] @ [W;b] -> out rows
"""

import sys

for _p in ("/opt/trn_rl_repo",):
    if _p not in sys.path:
        sys.path.insert(0, _p)

import numpy as np

import concourse.bacc as bacc
import concourse.mybir as mybir
import concourse.tile as tile

P = 128
F = 64            # feature dim
TC = 16           # fb_pad row padding unit (P*TC)
WN = 64           # nodes per window
GW = 4            # windows per group (gather-call batching)
CT = 8            # max tiles (1024 idx) per gather call
NQ = 4            # SWDGE queues
NCORES = 8

f32 = mybir.dt.float32
bf16 = mybir.dt.bfloat16
i32 = mybir.dt.int32
i16 = mybir.dt.int16
AX = mybir.AxisListType
OP = mybir.AluOpType
ACTF = mybir.ActivationFunctionType


# ----------------------------------------------------------------------------
# device program
# ----------------------------------------------------------------------------

def emit_program(tc, ins, outs, cfg):
    nc = tc.nc
    NPC = cfg["NPC"]            # nodes per core (multiple of WN)
    NB_pad = cfg["NB_pad"]
    H = cfg["H"]
    NT = cfg["NT"]              # total tiles
    plan = cfg["plan"]          # list per group: dict(calls, windows)
    fb_tab = ins["fb_tab"]
    faT = ins["faT"]            # [F+1, NPC] f32 (row F = ones)
    a1rep = ins["a1rep"]        # [F+1, P]   f32 (col-replicated a1 | ba2)
    wvec = ins["wvec"]          # [P, F]     f32 (Wa2 replicated)
    wb65 = ins["wb65"]          # [F+1, F]   f32 (W rows | b row)
    gidx = ins["gidx"]          # [P, NT*8]  i16
    soff = ins["soff"]          # [P, NT]    f32 (src offset in window, -1 pad)
    out = outs["out"]           # [NPC, F]   f32

    with (
        tc.tile_pool(name="const", bufs=1) as cpool,
        tc.tile_pool(name="main", bufs=cfg.get("rowsbufs", 2)) as mpool,
        tc.tile_pool(name="flsh", bufs=2) as fpool,
        tc.tile_pool(name="psum", bufs=3, space="PSUM") as psum,
        tc.tile_pool(name="psfl", bufs=1, space="PSUM") as psf,
    ):
        # ---------------- constants -----------------------------------
        # misc: [identity(128) | iota row(WN) | zeros(1)] from host
        misc = ins["misc"]
        misc_t = cpool.tile([P, P + WN + 1], f32)
        nc.sync.dma_start(out=misc_t[:], in_=misc)
        identb = cpool.tile([P, P], bf16)
        nc.vector.tensor_copy(out=identb[:], in_=misc_t[:, 0:P])
        iof = misc_t[:, P:P + WN]
        zb = misc_t[:, P + WN:P + WN + 1]
        wvec_t = cpool.tile([P, F], f32)
        nc.sync.dma_start(out=wvec_t[:], in_=wvec)
        w2b = cpool.tile([P, F], bf16)
        nc.vector.tensor_copy(out=w2b[:], in_=wvec_t[:])
        a1_t = cpool.tile([F + 1, P], f32)
        nc.sync.dma_start(out=a1_t[:], in_=a1rep)
        faT_t = cpool.tile([F + 1, NPC], f32)
        nc.sync.dma_start(out=faT_t[:], in_=faT)
        wb_t = cpool.tile([F + 1, F], f32)
        nc.sync.dma_start(out=wb_t[:], in_=wb65)
        wbb = cpool.tile([F + 1, F], bf16)
        nc.vector.tensor_copy(out=wbb[:], in_=wb_t[:])
        if not cfg.get("skipload"):
            soff_t = cpool.tile([P, NT], f32)
            nc.sync.dma_start(out=soff_t[:], in_=soff)
            gidx_t = cpool.tile([P, NT * 8], i16)
            nc.sync.dma_start(out=gidx_t[:], in_=gidx)

        # c1[n] = fa[n]@a1 + ba2, broadcast to all partitions via PE
        c1b = cpool.tile([P, NPC], bf16)
        CW = 512
        with tc.tile_pool(name="c1ps", bufs=1, space="PSUM") as cps:
            for j in range(-(-NPC // CW) if not cfg.get("skipc1") else 0):
                cw = min(CW, NPC - j * CW)
                ps = cps.tile([P, CW], f32, tag="c1ps")
                nc.tensor.matmul(out=ps[:, 0:cw], lhsT=a1_t[:],
                                 rhs=faT_t[:, j * CW:j * CW + cw],
                                 start=True, stop=True)
                nc.vector.tensor_copy(out=c1b[:, j * CW:j * CW + cw],
                                      in_=ps[:, 0:cw])

        # ---------------- main: per-group gather/score, per-window mm --
        # raw f32 fb rows are gathered directly (256B descriptors); the
        # bf16 cast, the ones column, and q = fb @ Wa2 are made per group.
        halfA = fb_tab[0:H, :]
        halfB = fb_tab[H:2 * H, :]
        TGmax = cfg["TGmax"]
        qrr = [0]

        stage = cfg.get("stage", 3)           # 1=gather, 2=+score, 3=all
        for g, gp in enumerate(plan if stage >= 1 else []):
            # per-call rotating tiles: calls chain only through the pool,
            # so gather DMAs overlap across calls instead of serializing
            call_tiles = []
            for ci, (t0, ntl, half, gt0) in enumerate(gp["calls"]):
                rows = mpool.tile([P, CT * F], f32, tag="rows", bufs=6)
                rows3 = rows[:].rearrange("p (t f) -> p t f", f=F)
                nc.gpsimd.dma_gather(
                    out_ap=rows3[:, 0:ntl, :],
                    in_ap=halfA if half == 0 else halfB,
                    idxs_ap=gidx_t[:, gt0 * 8:(gt0 + ntl) * 8],
                    num_idxs=ntl * P,
                    num_idxs_reg=ntl * P,
                    elem_size=F,
                    queue_num=qrr[0] % NQ,
                )
                qrr[0] += 1
                if stage < 2:
                    continue
                # rb = [bf16(fb) | 1];  q[e, t] = sum_f rb_f * Wa2_f
                rb = mpool.tile([P, CT * (F + 1)], bf16, tag="rb", bufs=8)
                rb3 = rb[:].rearrange("p (t f) -> p t f", f=F + 1)
                nc.vector.tensor_copy(out=rb3[:, 0:ntl, 0:F],
                                      in_=rows3[:, 0:ntl, :])
                nc.vector.memset(rb3[:, 0:ntl, F:F + 1], 1.0)
                prod = mpool.tile([P, CT * F], bf16, tag="prod", bufs=3)
                prod3 = prod[:].rearrange("p (t f) -> p t f", f=F)
                nc.vector.tensor_tensor(
                    out=prod3[:, 0:ntl, :], in0=rb3[:, 0:ntl, 0:F],
                    in1=w2b[:, None, :].to_broadcast([P, ntl, F]),
                    op=OP.mult,
                )
                q_t = mpool.tile([P, CT], f32, tag="q", bufs=3)
                nc.vector.tensor_reduce(out=q_t[:, 0:ntl],
                                        in_=prod3[:, 0:ntl, :],
                                        axis=AX.X, op=OP.add)
                # s[e, t, n] = c1[win(t)*WN + n] + q[e, t]
                s_t = mpool.tile([P, CT * WN], bf16, tag="s", bufs=3)
                s3 = s_t[:].rearrange("p (t n) -> p t n", n=WN)
                for (lt0, nn, w) in gp["cwsegs"][ci]:
                    nc.vector.tensor_tensor(
                        out=s3[:, lt0:lt0 + nn, :],
                        in0=c1b[:, w * WN:(w + 1) * WN][:, None, :]
                            .to_broadcast([P, nn, WN]),
                        in1=q_t[:, lt0:lt0 + nn, None]
                            .to_broadcast([P, nn, WN]),
                        op=OP.add,
                    )
                # score = max(e^s, 0.1 e^s + 0.9)  (== exp(elu(s, 0.1)))
                t_t = mpool.tile([P, CT * WN], bf16, tag="t", bufs=3)
                nc.scalar.activation(t_t[:, 0:ntl * WN], s_t[:, 0:ntl * WN],
                                     ACTF.Exp, bias=zb, scale=1.0)
                v_t = mpool.tile([P, CT * WN], bf16, tag="v", bufs=3)
                nc.vector.tensor_scalar(
                    out=v_t[:, 0:ntl * WN], in0=t_t[:, 0:ntl * WN],
                    scalar1=0.1, scalar2=0.9, op0=OP.mult, op1=OP.add,
                )
                nc.vector.tensor_tensor(
                    out=t_t[:, 0:ntl * WN], in0=t_t[:, 0:ntl * WN],
                    in1=v_t[:, 0:ntl * WN], op=OP.max,
                )
                # Seg[e, t, n] = [soff[e, t] == n] * score[e, t, n]
                seg = mpool.tile([P, CT * WN], bf16, tag="seg", bufs=8)
                seg3 = seg[:].rearrange("p (t n) -> p t n", n=WN)
                nc.vector.tensor_tensor(
                    out=seg3[:, 0:ntl, :],
                    in0=soff_t[:, gt0:gt0 + ntl][:, :, None]
                        .to_broadcast([P, ntl, WN]),
                    in1=iof[:, None, :].to_broadcast([P, ntl, WN]),
                    op=OP.is_equal,
                )
                nc.vector.tensor_tensor(
                    out=seg[:, 0:ntl * WN], in0=seg[:, 0:ntl * WN],
                    in1=t_t[:, 0:ntl * WN], op=OP.mult,
                )
                call_tiles.append((seg3, rb3))

            if stage < 3:
                continue
            for wd in gp["windows"]:
                w = wd["w"]
                tl = wd["ctiles"]             # list of (call_idx, local_t)
                pw = psum.tile([WN, F + 1], f32, tag="pw", bufs=4)
                for i, (ci, lt) in enumerate(tl):
                    seg3, rb3 = call_tiles[ci]
                    nc.tensor.matmul(out=pw[:], lhsT=seg3[:, lt, :],
                                     rhs=rb3[:, lt, 0:F + 1],
                                     start=(i == 0), stop=(i == len(tl) - 1))
                # flush: h = G/den_safe; out = [hT | den>0] @ [W; b]
                acc = fpool.tile([WN, F + 1], f32, tag="acc")
                nc.vector.tensor_copy(out=acc[:], in_=pw[:])
                den = acc[:, F:F + 1]
                dz = fpool.tile([WN, 1], f32, tag="dz")
                nc.vector.tensor_scalar(out=dz[:], in0=den, scalar1=0.0,
                                        scalar2=None, op0=OP.is_equal)
                ds = fpool.tile([WN, 1], f32, tag="ds")
                nc.vector.tensor_tensor(out=ds[:], in0=den, in1=dz[:],
                                        op=OP.add)
                rec = fpool.tile([WN, 1], f32, tag="rec")
                nc.vector.reciprocal(rec[:], ds[:])
                hm = fpool.tile([WN, F + 1], bf16, tag="hm")
                nc.vector.tensor_tensor(
                    out=hm[:, 0:F], in0=acc[:, 0:F],
                    in1=rec[:, 0:1].to_broadcast([WN, F]), op=OP.mult,
                )
                nc.vector.tensor_scalar(out=hm[:, F:F + 1], in0=dz[:],
                                        scalar1=-1.0, scalar2=1.0,
                                        op0=OP.mult, op1=OP.add)
                tp = psf.tile([F + 1, WN], bf16, tag="tp")
                nc.tensor.transpose(out=tp[:], in_=hm[:],
                                    identity=identb[0:WN, 0:WN])
                lt = fpool.tile([F + 1, WN], bf16, tag="lt")
                nc.vector.tensor_copy(out=lt[:], in_=tp[:])
                po = psf.tile([WN, F], f32, tag="po")
                nc.tensor.matmul(out=po[:], lhsT=lt[:], rhs=wbb[:],
                                 start=True, stop=True)
                ob = fpool.tile([WN, F], f32, tag="ob")
                nc.vector.tensor_copy(out=ob[:], in_=po[:])
                nc.sync.dma_start(out=out[w * WN:(w + 1) * WN, :], in_=ob[:])


# ----------------------------------------------------------------------------
# host-side preparation (index plumbing only, plus tiny param derivation)
# ----------------------------------------------------------------------------

def prep_inputs(feature_a, feature_b, W, b, a_vec, edges, node_num_a,
                ncores=NCORES):
    fa = np.asarray(feature_a, np.float32)
    fb = np.asarray(feature_b, np.float32)
    W = np.asarray(W, np.float32)
    b = np.asarray(b, np.float32)
    a_vec = np.asarray(a_vec, np.float32).reshape(-1)
    edges = np.asarray(edges)
    NA = int(node_num_a)
    NB = fb.shape[0]
    assert fb.shape[1] == F and fa.shape[1] == F

    a1 = a_vec[:F]
    a2 = a_vec[F:]
    Wa2 = (W @ a2).astype(np.float32)
    ba2 = float(b @ a2)

    NB_pad = -(-NB // (P * TC)) * (P * TC)
    H = NB_pad // 2
    assert H < 32768
    fb_pad = np.zeros((NB_pad, F), np.float32)
    fb_pad[:NB] = fb

    src = edges[:, 0].astype(np.int64)
    dst = edges[:, 1].astype(np.int64)

    NPC = -(-NA // (ncores * WN)) * WN
    NW = NPC // WN
    NG = -(-NW // GW)

    # per-core, per-(window, half) edge lists (sorted by src)
    order = np.lexsort((dst, src))
    ssrc = src[order]
    sdst = dst[order]
    shalf = (sdst >= H).astype(np.int64)

    core_of = ssrc // NPC
    win_of = (ssrc % NPC) // WN
    seg_key = ((core_of * NW + win_of) * 2 + shalf)
    seg_order = np.argsort(seg_key, kind="stable")
    k_sorted = seg_key[seg_order]
    e_src = ssrc[seg_order]
    e_dst = sdst[seg_order]
    n_segs = ncores * NW * 2
    cnt = np.bincount(k_sorted, minlength=n_segs).reshape(ncores, NW, 2)
    segoff = np.zeros(n_segs + 1, np.int64)
    np.cumsum(cnt.reshape(-1), out=segoff[1:])

    # uniform tile counts across cores (SPMD: one program for all)
    tcnt = -(-cnt // P)                       # [ncores, NW, 2]
    tcnt_u = tcnt.max(axis=0)                 # [NW, 2]

    # group plan (same for every core)
    plan = []
    NT = 0
    TGmax = 0
    for g in range(NG):
        ws = list(range(g * GW, min((g + 1) * GW, NW)))
        gt0 = NT
        calls = []
        wsegs = []
        windows = {w: [] for w in ws}
        t_in_g = 0
        for half in (0, 1):
            run_t0, run_gt0, run_len = t_in_g, NT, 0

            def flush_run():
                nonlocal run_len, run_t0, run_gt0
                while run_len > 0:
                    n = min(run_len, CT)
                    calls.append((run_t0, n, half, run_gt0))
                    run_t0 += n
                    run_gt0 += n
                    run_len -= n

            for w in ws:
                ntl = int(tcnt_u[w, half])
                if ntl == 0:
                    continue
                wsegs.append((t_in_g, ntl, w))
                windows[w].extend(range(t_in_g, t_in_g + ntl))
                t_in_g += ntl
                NT += ntl
                run_len += ntl
            flush_run()
        TGmax = max(TGmax, t_in_g)

        def to_call(t):
            for ci, (t0, ntl, half, gt0_) in enumerate(calls):
                if t0 <= t < t0 + ntl:
                    return ci, t - t0
            raise AssertionError(t)

        cwsegs = [[] for _ in calls]
        for (t0w, ntlw, w) in wsegs:
            t = t0w
            while t < t0w + ntlw:
                ci, lt = to_call(t)
                n = min(t0w + ntlw - t, calls[ci][1] - lt)
                cwsegs[ci].append((lt, n, w))
                t += n
        plan.append(dict(
            TG=t_in_g, gt0=gt0, calls=calls, wsegs=wsegs, cwsegs=cwsegs,
            windows=[dict(w=w, tiles=windows[w],
                          ctiles=[to_call(t) for t in windows[w]])
                     for w in ws if windows[w]],
        ))

    # per-core padded idx / srcoff arrays following the uniform layout
    in_maps = []
    for c in range(ncores):
        idx_all = np.zeros(NT * P, np.int16)
        sof_all = np.full(NT * P, -1.0, np.float32)
        t_base = 0
        for g in range(NG):
            ws = range(g * GW, min((g + 1) * GW, NW))
            for half in (0, 1):
                for w in ws:
                    ntl = int(tcnt_u[w, half])
                    if ntl == 0:
                        continue
                    s = (c * NW + w) * 2 + half
                    lo, hi = segoff[s], segoff[s + 1]
                    ne = hi - lo
                    assert ne <= ntl * P
                    sl = slice(t_base * P, t_base * P + ne)
                    d = e_dst[lo:hi] - (H if half else 0)
                    idx_all[sl] = d.astype(np.int16)
                    sof_all[sl] = (e_src[lo:hi] % NPC - w * WN)
                    t_base += ntl
        assert t_base == NT
        gidx = np.tile(idx_all.reshape(NT * 8, 16).T, (8, 1))
        soff = np.ascontiguousarray(sof_all.reshape(NT, P).T)

        faT65 = np.zeros((F + 1, NPC), np.float32)
        n_real = min(NPC, NA - c * NPC)
        if n_real > 0:
            faT65[0:F, 0:n_real] = fa[c * NPC:c * NPC + n_real].T
        faT65[F, :] = 1.0

        a1rep = np.zeros((F + 1, P), np.float32)
        a1rep[0:F, :] = a1[:, None]
        a1rep[F, :] = ba2

        wvec = np.tile(Wa2[None, :], (P, 1)).astype(np.float32)
        wb65 = np.zeros((F + 1, F), np.float32)
        wb65[0:F] = W
        wb65[F] = b

        misc = np.zeros((P, P + WN + 1), np.float32)
        misc[:, 0:P] = np.eye(P, dtype=np.float32)
        misc[:, P:P + WN] = np.arange(WN, dtype=np.float32)[None, :]
        in_maps.append(dict(
            fb_tab=fb_pad, faT=faT65, a1rep=a1rep, wvec=wvec, wb65=wb65,
            gidx=np.ascontiguousarray(gidx),
            soff=soff, misc=misc,
        ))

    cfg = dict(NPC=NPC, NB_pad=NB_pad, H=H, NT=NT, TGmax=TGmax, plan=plan,
               NA=NA, NW=NW)
    return in_maps, cfg


def build_bass(cfg, ncores=NCORES):
    nc = bacc.Bacc("TRN2", target_bir_lowering=False, debug=False,
                   enable_asserts=False, num_devices=ncores,
                   num_swdge_queues=NQ)
    ins = dict(
        fb_tab=nc.dram_tensor("fb_tab", [cfg["NB_pad"], F], f32,
                              kind="ExternalInput").ap(),
        faT=nc.dram_tensor("faT", [F + 1, cfg["NPC"]], f32,
                           kind="ExternalInput").ap(),
        a1rep=nc.dram_tensor("a1rep", [F + 1, P], f32,
                             kind="ExternalInput").ap(),
        wvec=nc.dram_tensor("wvec", [P, F], f32, kind="ExternalInput").ap(),
        wb65=nc.dram_tensor("wb65", [F + 1, F], f32,
                            kind="ExternalInput").ap(),
        gidx=nc.dram_tensor("gidx", [P, cfg["NT"] * 8], i16,
                            kind="ExternalInput").ap(),
        misc=nc.dram_tensor("misc", [P, P + WN + 1], f32,
                            kind="ExternalInput").ap(),
        soff=nc.dram_tensor("soff", [P, cfg["NT"]], f32,
                            kind="ExternalInput").ap(),
    )
    outs = dict(
        out=nc.dram_tensor("out", [cfg["NPC"], F], f32,
                           kind="ExternalOutput").ap(),
    )
    with tile.TileContext(nc) as tc:
        emit_program(tc, ins, outs, cfg)
    nc.compile()
    return nc


# ----------------------------------------------------------------------------
# entry point
# ----------------------------------------------------------------------------

def assemble_output(results, cfg):
    outs = [r["out"] for r in results]
    return np.concatenate(outs, axis=0)[:cfg["NA"]].astype(np.float32)


def kernel_with_results(trace=False, **inputs):
    from concourse import bass_utils

    in_maps, cfg = prep_inputs(**inputs)
    nc = build_bass(cfg)
    res = bass_utils.run_bass_kernel_spmd(
        nc, in_maps, core_ids=list(range(NCORES)), trace=trace,
    )
    return assemble_output(res.results, cfg), res


def kernel(**inputs):
    return kernel_with_results(trace=False, **inputs)[0]


if __name__ == "__main__":
    np.random.seed(0)
    NA = NB = 50000
    E = 800000
    ins = dict(
        feature_a=np.random.randn(NA, F).astype(np.float32),
        feature_b=np.random.randn(NB, F).astype(np.float32),
        W=(np.random.randn(F, F) / 8).astype(np.float32),
        b=np.zeros(F, np.float32),
        a_vec=(np.random.randn(2 * F, 1) * 0.05).astype(np.float32),
        edges=np.stack([np.random.randint(0, NA, E),
                        np.random.randint(0, NB, E)], 1).astype(np.int64),
        node_num_a=NA,
    )
    out = kernel(**ins)
    print(out.shape, out.dtype)
